# revision 1
# baseline (speedup 1.0000x reference)
"""Trainium2 Bass kernel for nn_Network_73212012528289 (histogram_binning).

Pipeline (8 NeuronCores, pure data parallelism over the batch axis):
  - Bass kernel (per core, one batch of 256 channels):
      ij = center*100 + (boxsum3x3(center)-center)/8  (reference's exact f32
      op order), full 16384-wide bitonic sort of each channel row, then
      adjacent-equal duplicate stats: count + top-8 flag positions.
  - Host: reconstruct the reference's exact `counts` array from the duplicate
    positions (pure integer work), then run the reference's own jax ops
    (divide/where/log/mul/sum/top_k) on CPU XLA for bit-exact entropies and
    channel selection.
  - Gather selected channels.

The kernel is self-contained and input-agnostic (handles any multiplicity
structure; rows with >8 duplicate pairs fall back to an exact host path).
"""
import os
import subprocess
import sys
import tempfile

import numpy as np

from concourse import bass, mybir
from concourse.alu_op_type import AluOpType

H = W = 112
M = H * W            # 12544
PW = 114
PM = PW * PW         # 12996
N = 16384            # sort width (pow2 >= M)
CBLK = 128
NBLK = 2             # 256 channels / 128 per block
BIG = 3.0e38         # finite sort sentinel > any |ij|
B = 8
C = 256
K = 192              # int(0.75 * 256)
DENOM = float((H + 2) * (W + 2))


def _bitonic_passes(n):
    k = 2
    while k <= n:
        j = k // 2
        while j >= 1:
            yield k, j
            j //= 2
        k *= 2


def _build_kernel_a():
    nc = bass.Bass("TRN2", target_bir_lowering=False, debug=False)
    x = nc.dram_tensor("x", [2 * CBLK, H, W], mybir.dt.float32, kind="ExternalInput")
    nd_o = nc.dram_tensor("nd", [2 * CBLK, 1], mybir.dt.float32, kind="ExternalOutput")
    dpos_o = nc.dram_tensor("dpos", [2 * CBLK, 8], mybir.dt.float32, kind="ExternalOutput")

    with (
        nc.sbuf_tensor([128, N], mybir.dt.float32) as b1,
        nc.sbuf_tensor([128, N], mybir.dt.float32) as b2,
        nc.sbuf_tensor([128, M - 1], mybir.dt.float32) as iota_t,
        nc.sbuf_tensor([128, 1], mybir.dt.float32) as ndt,
        nc.sbuf_tensor([128, 8], mybir.dt.float32) as dpt,
        nc.semaphore() as dma_sem,
        nc.semaphore() as p_sem,
        nc.semaphore() as v_sem,
        nc.Block() as block,
    ):
        p_ct = [0]

        @block.gpsimd
        def _(g):
            g.iota(iota_t[:], pattern=[[1, M - 1]], base=1, channel_multiplier=0,
                   allow_small_or_imprecise_dtypes=True)
            g.sem_inc(p_sem, 1)
            for blk in range(NBLK):
                # pad-zero memset of b2 must complete before the image DMA
                g.wait_ge(v_sem, blk * 2 + 1)
                g.dma_start(
                    b2[:, :PM].rearrange("p (h w) -> p h w", w=PW)[:, 1:113, 1:113],
                    x[blk * CBLK:(blk + 1) * CBLK],
                ).then_inc(dma_sem, 16)
                g.wait_ge(v_sem, blk * 2 + 2)
                g.dma_start(nd_o[blk * CBLK:(blk + 1) * CBLK], ndt[:]).then_inc(dma_sem, 16)
                g.dma_start(dpos_o[blk * CBLK:(blk + 1) * CBLK], dpt[:]).then_inc(dma_sem, 16)

        @block.vector
        def _(v):
            def step(ins):
                p_ct[0] += 1
                ins.then_inc(p_sem, 1)
                return ins

            def fence():
                v.wait_ge(p_sem, p_ct[0])

            v.wait_ge(p_sem, 1)       # iota done
            p_ct[0] += 1              # gpsimd's inc
            for blk in range(NBLK):
                # previous block's output DMAs must be done (ndt/dpt reuse)
                v.wait_ge(dma_sem, blk * 48)
                step(nc.vector.memset(b2[:, :PM], 0.0))
                fence()
                nc.vector.engine_nop().then_inc(v_sem, 1)      # allow image DMA
                v.wait_ge(dma_sem, blk * 48 + 16)               # image landed

                pad = b2[:, :PM].rearrange("p (h w) -> p h w", w=PW)
                sview = b1[:, :M].rearrange("p (h w) -> p h w", w=W)
                # s = 0; s += p[dy:dy+H, dx:dx+W] in the reference's order
                step(nc.vector.memset(b1[:, :M], 0.0))
                fence()
                for dy in range(3):
                    for dx in range(3):
                        t = pad[:, dy:dy + H, dx:dx + W]
                        step(nc.vector.tensor_tensor(sview, sview, t, AluOpType.add))
                        fence()
                interior = pad[:, 1:113, 1:113]
                step(nc.vector.tensor_tensor(sview, sview, interior, AluOpType.subtract))
                fence()
                step(nc.vector.tensor_scalar_mul(b1[:, :M], b1[:, :M], 0.125))
                fence()
                step(nc.vector.scalar_tensor_tensor(sview, interior, 100.0, sview,
                                                    AluOpType.mult, AluOpType.add))
                fence()
                step(nc.vector.memset(b1[:, M:], BIG))
                fence()

                # bitonic sort b1 -> ... -> b2 (105 ping-pong passes)
                bufs = [b1, b2]
                cur = 0
                for k, j in _bitonic_passes(N):
                    bi, bo = bufs[cur], bufs[cur ^ 1]
                    if k < N:
                        a = N // (2 * k)
                        g_ = k // (2 * j)
                        vi = bi[:].rearrange("p (a h g s e) -> p a h g s e", a=a, h=2, g=g_, s=2, e=j)
                        vo = bo[:].rearrange("p (a h g s e) -> p a h g s e", a=a, h=2, g=g_, s=2, e=j)
                        step(nc.vector.tensor_tensor(vo[:, :, 0, :, 0, :], vi[:, :, 0, :, 0, :], vi[:, :, 0, :, 1, :], AluOpType.min))
                        step(nc.vector.tensor_tensor(vo[:, :, 0, :, 1, :], vi[:, :, 0, :, 0, :], vi[:, :, 0, :, 1, :], AluOpType.max))
                        step(nc.vector.tensor_tensor(vo[:, :, 1, :, 0, :], vi[:, :, 1, :, 0, :], vi[:, :, 1, :, 1, :], AluOpType.max))
                        step(nc.vector.tensor_tensor(vo[:, :, 1, :, 1, :], vi[:, :, 1, :, 0, :], vi[:, :, 1, :, 1, :], AluOpType.min))
                    else:
                        g_ = k // (2 * j)
                        vi = bi[:].rearrange("p (g s e) -> p g s e", g=g_, s=2, e=j)
                        vo = bo[:].rearrange("p (g s e) -> p g s e", g=g_, s=2, e=j)
                        step(nc.vector.tensor_tensor(vo[:, :, 0, :], vi[:, :, 0, :], vi[:, :, 1, :], AluOpType.min))
                        step(nc.vector.tensor_tensor(vo[:, :, 1, :], vi[:, :, 0, :], vi[:, :, 1, :], AluOpType.max))
                    fence()
                    cur ^= 1
                srt = bufs[cur]
                scr = bufs[cur ^ 1]

                # duplicate stats
                step(nc.vector.tensor_tensor(scr[:, :M - 1], srt[:, :M - 1], srt[:, 1:M], AluOpType.is_equal))
                fence()
                step(nc.vector.reduce_sum(ndt[:], scr[:, :M - 1], axis=mybir.AxisListType.X))
                step(nc.vector.tensor_tensor(scr[:, :M - 1], scr[:, :M - 1], iota_t[:], AluOpType.mult))
                fence()
                step(nc.vector.max(dpt[:], scr[:, :M - 1]))
                fence()
                nc.vector.engine_nop().then_inc(v_sem, 1)

    return nc


def _ij_numpy(img):
    """Bit-exact f32 replication of the reference's ij computation."""
    Bv = img.shape[0]
    p = np.pad(img, [(0, 0)] * (img.ndim - 2) + [(1, 1), (1, 1)]).astype(np.float32)
    s = np.zeros_like(img)
    for dy in range(3):
        for dx in range(3):
            s = (s + p[..., dy:dy + H, dx:dx + W]).astype(np.float32)
    mean = ((s - img) / np.float32(8.0)).astype(np.float32)
    return (img * np.float32(100.0) + mean).astype(np.float32)


def _counts_from_flag_positions(pos_list):
    """counts row (length M, f32) from ascending duplicate-flag positions."""
    counts = np.ones(M, np.float32)
    if len(pos_list) == 0:
        return counts
    pos = np.asarray(pos_list)
    groups = np.split(pos, np.flatnonzero(np.diff(pos) != 1) + 1)
    nflags_before = 0
    for g in groups:
        counts[int(g[0]) - nflags_before] = len(g) + 1
        nflags_before += len(g)
    counts[M - len(pos):] = 0.0
    return counts


_TAIL_SRC = r"""
import sys
import numpy as np
import jax
import jax.numpy as jnp
cfile, efile, ifile = sys.argv[1], sys.argv[2], sys.argv[3]
counts = jnp.asarray(np.load(cfile))
denom = jnp.float32(%f)
pr = counts / denom
p_safe = jnp.where(counts > 0, pr, 1.0)
h = jnp.where(counts > 0, -pr * (jnp.log(p_safe) / jnp.log(2.0)), 0.0)
ent = jnp.sum(h, axis=1)
ent = np.asarray(ent).reshape(%d, %d)
_, idx = jax.lax.top_k(jnp.asarray(ent), %d)
np.save(efile, ent)
np.save(ifile, np.asarray(idx))
print("TAIL_BACKEND", jax.default_backend())
""" % (DENOM, B, C, K)


def _run_tail_cpu(counts2d):
    """Run the reference's counts->ent->topk jax ops on CPU XLA (bit-exact).

    Returns (ent [B,C] f32, idx [B,K] int32). Falls back to in-process jax
    if the CPU subprocess cannot be launched.
    """
    import jax  # noqa: PLC0415

    site_dir = os.path.dirname(os.path.dirname(os.path.abspath(jax.__file__)))
    with tempfile.TemporaryDirectory() as td:
        cfile = os.path.join(td, "counts.npy")
        efile = os.path.join(td, "ent.npy")
        ifile = os.path.join(td, "idx.npy")
        np.save(cfile, counts2d)
        env = dict(os.environ)
        env["JAX_PLATFORMS"] = "cpu"
        env["TRN_TERMINAL_POOL_IPS"] = ""   # disable axon boot in sitecustomize
        env["PYTHONPATH"] = site_dir
        try:
            r = subprocess.run(
                [sys.executable, "-c", _TAIL_SRC, cfile, efile, ifile],
                env=env, capture_output=True, text=True, timeout=600,
            )
            if r.returncode == 0 and os.path.exists(ifile):
                return np.load(efile), np.load(ifile)
            sys.stderr.write("tail subprocess failed:\n" + r.stdout + r.stderr)
        except Exception as e:  # pragma: no cover
            sys.stderr.write(f"tail subprocess error: {e}\n")
        # fallback: in-process jax (whatever backend is active)
        import jax.numpy as jnp  # noqa: PLC0415
        counts = jnp.asarray(counts2d)
        denom = jnp.float32(DENOM)
        pr = counts / denom
        p_safe = jnp.where(counts > 0, pr, 1.0)
        hh = jnp.where(counts > 0, -pr * (jnp.log(p_safe) / jnp.log(2.0)), 0.0)
        ent = np.asarray(jnp.sum(hh, axis=1)).reshape(B, C)
        _, idx = jax.lax.top_k(jnp.asarray(ent), K)
        return ent, np.asarray(idx)


def kernel(img: np.ndarray) -> np.ndarray:
    from concourse.bass_utils import run_bass_kernel_spmd  # noqa: PLC0415

    img = np.ascontiguousarray(np.asarray(img, dtype=np.float32))
    assert img.shape == (B, C, H, W), img.shape

    # ---- device: duplicate stats per (batch, channel) row --------------
    nc = _build_kernel_a()
    in_maps = [{"x": np.ascontiguousarray(img[b])} for b in range(B)]
    res = run_bass_kernel_spmd(nc, in_maps, list(range(B)))
    nd = np.stack([res.results[b]["nd"][:, 0] for b in range(B)])          # [B, C]
    dpos = np.stack([res.results[b]["dpos"] for b in range(B)])            # [B, C, 8]

    # ---- host: exact counts reconstruction -----------------------------
    nd_i = nd.astype(np.int64)
    counts2d = np.ones((B * C, M), np.float32)
    ij_host = None
    for b in range(B):
        for c in range(C):
            r = b * C + c
            t = nd_i[b, c]
            if t == 0:
                continue
            if t <= 8:
                p = np.sort(dpos[b, c][dpos[b, c] > 0]).astype(np.int64) - 1
                ok = len(p) == t
            else:
                ok = False
            if not ok:
                # exact host fallback for overflow rows (>8 duplicate pairs)
                if ij_host is None:
                    ij_host = _ij_numpy(img).reshape(B * C, M)
                srow = np.sort(ij_host[r])
                p = np.flatnonzero(srow[1:] == srow[:-1])
            counts2d[r] = _counts_from_flag_positions(p)

    # ---- host: reference's jax ops on CPU XLA (bit-exact ent + top_k) --
    _, idx = _run_tail_cpu(counts2d)

    # ---- gather selected channels (pure indexing, exact) ---------------
    out = np.take_along_axis(img, idx.astype(np.int64)[:, :, None, None], axis=1)
    return np.ascontiguousarray(out)


if __name__ == "__main__":
    img = np.load("/tmp/cpu_img.npy")
    out = kernel(img=img)
    exp = np.load("/tmp/cpu_refout.npy")
    print("exact match vs CPU reference:", np.array_equal(out, exp))


# revision 2
# speedup vs baseline: 19.2832x; 19.2832x over previous
"""Trainium2 Bass kernel for nn_Network_73212012528289 (histogram_binning).

Pipeline (8 NeuronCores, pure data parallelism over the batch axis):
  - Bass kernel (per core, one batch of 256 channels):
      ij = center*100 + (boxsum3x3(center)-center)/8  (reference's exact f32
      op order), full 16384-wide bitonic sort of each channel row, then
      adjacent-equal duplicate stats: count + top-8 flag positions.
  - Host: reconstruct the reference's exact `counts` array from the duplicate
    positions (pure integer work), then run the reference's own jax ops
    (divide/where/log/mul/sum/top_k) on CPU XLA for bit-exact entropies and
    channel selection.
  - Gather selected channels.

The kernel is self-contained and input-agnostic (handles any multiplicity
structure; rows with >8 duplicate pairs fall back to an exact host path).
"""
import os
import subprocess
import sys
import tempfile

import numpy as np

from concourse import bass, mybir
from concourse.alu_op_type import AluOpType

H = W = 112
M = H * W            # 12544
PW = 114
PM = PW * PW         # 12996
N = 16384            # sort width (pow2 >= M)
CBLK = 128
NBLK = 2             # 256 channels / 128 per block
BIG = 3.0e38         # finite sort sentinel > any |ij|
B = 8
C = 256
K = 192              # int(0.75 * 256)
DENOM = float((H + 2) * (W + 2))


def _bitonic_passes(n):
    k = 2
    while k <= n:
        j = k // 2
        while j >= 1:
            yield k, j
            j //= 2
        k *= 2


def _build_kernel_a():
    nc = bass.Bass("TRN2", target_bir_lowering=False, debug=False)
    x = nc.dram_tensor("x", [2 * CBLK, H, W], mybir.dt.float32, kind="ExternalInput")
    nd_o = nc.dram_tensor("nd", [2 * CBLK, 1], mybir.dt.float32, kind="ExternalOutput")
    dpos_o = nc.dram_tensor("dpos", [2 * CBLK, 8], mybir.dt.float32, kind="ExternalOutput")

    with (
        nc.sbuf_tensor([128, N], mybir.dt.float32) as b1,
        nc.sbuf_tensor([128, N], mybir.dt.float32) as b2,
        nc.sbuf_tensor([128, M - 1], mybir.dt.float32) as iota_t,
        nc.sbuf_tensor([128, 1], mybir.dt.float32) as ndt,
        nc.sbuf_tensor([128, 8], mybir.dt.float32) as dpt,
        nc.semaphore() as dma_sem,
        nc.semaphore() as p_sem,
        nc.semaphore() as v_sem,
        nc.Block() as block,
    ):
        p_ct = [0]

        @block.gpsimd
        def _(g):
            g.iota(iota_t[:], pattern=[[1, M - 1]], base=1, channel_multiplier=0,
                   allow_small_or_imprecise_dtypes=True)
            g.sem_inc(p_sem, 1)
            for blk in range(NBLK):
                # pad-zero memset of b2 must complete before the image DMA
                g.wait_ge(v_sem, blk * 2 + 1)
                g.dma_start(
                    b2[:, :PM].rearrange("p (h w) -> p h w", w=PW)[:, 1:113, 1:113],
                    x[blk * CBLK:(blk + 1) * CBLK],
                ).then_inc(dma_sem, 16)
                g.wait_ge(v_sem, blk * 2 + 2)
                g.dma_start(nd_o[blk * CBLK:(blk + 1) * CBLK], ndt[:]).then_inc(dma_sem, 16)
                g.dma_start(dpos_o[blk * CBLK:(blk + 1) * CBLK], dpt[:]).then_inc(dma_sem, 16)

        @block.vector
        def _(v):
            def step(ins):
                p_ct[0] += 1
                ins.then_inc(p_sem, 1)
                return ins

            def fence():
                v.wait_ge(p_sem, p_ct[0])

            v.wait_ge(p_sem, 1)       # iota done
            p_ct[0] += 1              # gpsimd's inc
            for blk in range(NBLK):
                # previous block's output DMAs must be done (ndt/dpt reuse)
                v.wait_ge(dma_sem, blk * 48)
                step(nc.vector.memset(b2[:, :PM], 0.0))
                fence()
                nc.vector.engine_nop().then_inc(v_sem, 1)      # allow image DMA
                v.wait_ge(dma_sem, blk * 48 + 16)               # image landed

                pad = b2[:, :PM].rearrange("p (h w) -> p h w", w=PW)
                sview = b1[:, :M].rearrange("p (h w) -> p h w", w=W)
                # s = 0; s += p[dy:dy+H, dx:dx+W] in the reference's order
                step(nc.vector.memset(b1[:, :M], 0.0))
                fence()
                for dy in range(3):
                    for dx in range(3):
                        t = pad[:, dy:dy + H, dx:dx + W]
                        step(nc.vector.tensor_tensor(sview, sview, t, AluOpType.add))
                        fence()
                interior = pad[:, 1:113, 1:113]
                step(nc.vector.tensor_tensor(sview, sview, interior, AluOpType.subtract))
                fence()
                step(nc.vector.tensor_scalar_mul(b1[:, :M], b1[:, :M], 0.125))
                fence()
                step(nc.vector.scalar_tensor_tensor(sview, interior, 100.0, sview,
                                                    AluOpType.mult, AluOpType.add))
                fence()
                step(nc.vector.memset(b1[:, M:], BIG))
                fence()
                # b2's pad-image leftovers in [M:] must also be BIG: skipped
                # all-sentinel blocks below are never rewritten, so BOTH
                # ping-pong buffers' tails must hold the sentinel.
                step(nc.vector.memset(b2[:, M:], BIG))
                fence()

                # bitonic sort b1 -> ... -> b2 (105 ping-pong passes).
                # 2k-blocks that lie entirely in the sentinel tail are already
                # sorted (all equal) and are skipped: process only the first
                # ceil(M / 2k) blocks of each stage.
                bufs = [b1, b2]
                cur = 0
                for k, j in _bitonic_passes(N):
                    bi, bo = bufs[cur], bufs[cur ^ 1]
                    if k < N:
                        a = N // (2 * k)
                        a_cnt = min(a, -(-M // (2 * k)))
                        g_ = k // (2 * j)
                        vi = bi[:].rearrange("p (a h g s e) -> p a h g s e", a=a, h=2, g=g_, s=2, e=j)
                        vo = bo[:].rearrange("p (a h g s e) -> p a h g s e", a=a, h=2, g=g_, s=2, e=j)
                        step(nc.vector.tensor_tensor(vo[:, 0:a_cnt, 0, :, 0, :], vi[:, 0:a_cnt, 0, :, 0, :], vi[:, 0:a_cnt, 0, :, 1, :], AluOpType.min))
                        step(nc.vector.tensor_tensor(vo[:, 0:a_cnt, 0, :, 1, :], vi[:, 0:a_cnt, 0, :, 0, :], vi[:, 0:a_cnt, 0, :, 1, :], AluOpType.max))
                        step(nc.vector.tensor_tensor(vo[:, 0:a_cnt, 1, :, 0, :], vi[:, 0:a_cnt, 1, :, 0, :], vi[:, 0:a_cnt, 1, :, 1, :], AluOpType.max))
                        step(nc.vector.tensor_tensor(vo[:, 0:a_cnt, 1, :, 1, :], vi[:, 0:a_cnt, 1, :, 0, :], vi[:, 0:a_cnt, 1, :, 1, :], AluOpType.min))
                    else:
                        g_ = k // (2 * j)
                        vi = bi[:].rearrange("p (g s e) -> p g s e", g=g_, s=2, e=j)
                        vo = bo[:].rearrange("p (g s e) -> p g s e", g=g_, s=2, e=j)
                        step(nc.vector.tensor_tensor(vo[:, :, 0, :], vi[:, :, 0, :], vi[:, :, 1, :], AluOpType.min))
                        step(nc.vector.tensor_tensor(vo[:, :, 1, :], vi[:, :, 0, :], vi[:, :, 1, :], AluOpType.max))
                    fence()
                    cur ^= 1
                srt = bufs[cur]
                scr = bufs[cur ^ 1]

                # duplicate stats
                step(nc.vector.tensor_tensor(scr[:, :M - 1], srt[:, :M - 1], srt[:, 1:M], AluOpType.is_equal))
                fence()
                step(nc.vector.reduce_sum(ndt[:], scr[:, :M - 1], axis=mybir.AxisListType.X))
                step(nc.vector.tensor_tensor(scr[:, :M - 1], scr[:, :M - 1], iota_t[:], AluOpType.mult))
                fence()
                step(nc.vector.max(dpt[:], scr[:, :M - 1]))
                fence()
                nc.vector.engine_nop().then_inc(v_sem, 1)

    return nc


def _ij_numpy(img):
    """Bit-exact f32 replication of the reference's ij computation."""
    Bv = img.shape[0]
    p = np.pad(img, [(0, 0)] * (img.ndim - 2) + [(1, 1), (1, 1)]).astype(np.float32)
    s = np.zeros_like(img)
    for dy in range(3):
        for dx in range(3):
            s = (s + p[..., dy:dy + H, dx:dx + W]).astype(np.float32)
    mean = ((s - img) / np.float32(8.0)).astype(np.float32)
    return (img * np.float32(100.0) + mean).astype(np.float32)


def _counts_from_flag_positions(pos_list):
    """counts row (length M, f32) from ascending duplicate-flag positions."""
    counts = np.ones(M, np.float32)
    if len(pos_list) == 0:
        return counts
    pos = np.asarray(pos_list)
    groups = np.split(pos, np.flatnonzero(np.diff(pos) != 1) + 1)
    nflags_before = 0
    for g in groups:
        counts[int(g[0]) - nflags_before] = len(g) + 1
        nflags_before += len(g)
    counts[M - len(pos):] = 0.0
    return counts


_TAIL_SRC = r"""
import sys
import numpy as np
import jax
import jax.numpy as jnp
cfile, efile, ifile = sys.argv[1], sys.argv[2], sys.argv[3]
counts = jnp.asarray(np.load(cfile))
denom = jnp.float32(%f)
pr = counts / denom
p_safe = jnp.where(counts > 0, pr, 1.0)
h = jnp.where(counts > 0, -pr * (jnp.log(p_safe) / jnp.log(2.0)), 0.0)
ent = jnp.sum(h, axis=1)
ent = np.asarray(ent).reshape(%d, %d)
_, idx = jax.lax.top_k(jnp.asarray(ent), %d)
np.save(efile, ent)
np.save(ifile, np.asarray(idx))
print("TAIL_BACKEND", jax.default_backend())
""" % (DENOM, B, C, K)


def _run_tail_cpu(counts2d):
    """Run the reference's counts->ent->topk jax ops on CPU XLA (bit-exact).

    Returns (ent [B,C] f32, idx [B,K] int32). Falls back to in-process jax
    if the CPU subprocess cannot be launched.
    """
    import jax  # noqa: PLC0415

    site_dir = os.path.dirname(os.path.dirname(os.path.abspath(jax.__file__)))
    with tempfile.TemporaryDirectory() as td:
        cfile = os.path.join(td, "counts.npy")
        efile = os.path.join(td, "ent.npy")
        ifile = os.path.join(td, "idx.npy")
        np.save(cfile, counts2d)
        env = dict(os.environ)
        env["JAX_PLATFORMS"] = "cpu"
        env["TRN_TERMINAL_POOL_IPS"] = ""   # disable axon boot in sitecustomize
        env["PYTHONPATH"] = site_dir
        try:
            r = subprocess.run(
                [sys.executable, "-c", _TAIL_SRC, cfile, efile, ifile],
                env=env, capture_output=True, text=True, timeout=600,
            )
            if r.returncode == 0 and os.path.exists(ifile):
                return np.load(efile), np.load(ifile)
            sys.stderr.write("tail subprocess failed:\n" + r.stdout + r.stderr)
        except Exception as e:  # pragma: no cover
            sys.stderr.write(f"tail subprocess error: {e}\n")
        # fallback: in-process jax (whatever backend is active)
        import jax.numpy as jnp  # noqa: PLC0415
        counts = jnp.asarray(counts2d)
        denom = jnp.float32(DENOM)
        pr = counts / denom
        p_safe = jnp.where(counts > 0, pr, 1.0)
        hh = jnp.where(counts > 0, -pr * (jnp.log(p_safe) / jnp.log(2.0)), 0.0)
        ent = np.asarray(jnp.sum(hh, axis=1)).reshape(B, C)
        _, idx = jax.lax.top_k(jnp.asarray(ent), K)
        return ent, np.asarray(idx)


def kernel(img: np.ndarray) -> np.ndarray:
    from concourse.bass_utils import run_bass_kernel_spmd  # noqa: PLC0415

    img = np.ascontiguousarray(np.asarray(img, dtype=np.float32))
    assert img.shape == (B, C, H, W), img.shape

    # ---- device: duplicate stats per (batch, channel) row --------------
    nc = _build_kernel_a()
    in_maps = [{"x": np.ascontiguousarray(img[b])} for b in range(B)]
    res = run_bass_kernel_spmd(nc, in_maps, list(range(B)))
    nd = np.stack([res.results[b]["nd"][:, 0] for b in range(B)])          # [B, C]
    dpos = np.stack([res.results[b]["dpos"] for b in range(B)])            # [B, C, 8]

    # ---- host: exact counts reconstruction -----------------------------
    nd_i = nd.astype(np.int64)
    counts2d = np.ones((B * C, M), np.float32)
    ij_host = None
    for b in range(B):
        for c in range(C):
            r = b * C + c
            t = nd_i[b, c]
            if t == 0:
                continue
            if t <= 8:
                p = np.sort(dpos[b, c][dpos[b, c] > 0]).astype(np.int64) - 1
                ok = len(p) == t
            else:
                ok = False
            if not ok:
                # exact host fallback for overflow rows (>8 duplicate pairs)
                if ij_host is None:
                    ij_host = _ij_numpy(img).reshape(B * C, M)
                srow = np.sort(ij_host[r])
                p = np.flatnonzero(srow[1:] == srow[:-1])
            counts2d[r] = _counts_from_flag_positions(p)

    # ---- host: reference's jax ops on CPU XLA (bit-exact ent + top_k) --
    _, idx = _run_tail_cpu(counts2d)

    # ---- gather selected channels (pure indexing, exact) ---------------
    out = np.take_along_axis(img, idx.astype(np.int64)[:, :, None, None], axis=1)
    return np.ascontiguousarray(out)


if __name__ == "__main__":
    img = np.load("/tmp/cpu_img.npy")
    out = kernel(img=img)
    exp = np.load("/tmp/cpu_refout.npy")
    print("exact match vs CPU reference:", np.array_equal(out, exp))
